# revision 19
# baseline (speedup 1.0000x reference)
"""Trainium2 Bass kernel for nn_CosineDistance (retrieval maxsim).

Reference computation:
    pano_n = l2norm(pano [64,64,128]);  sat_n = l2norm(sat [256,64,128])
    sim[a,b,i,j] = pano_n[a,i,:] . sat_n[b,j,:]
    out[a,b] = sim.max(axis=j).sum(axis=i)           -> [64, 256] fp32

Sharding: sat (b) axis split across 8 cores, 32 sats each. Each core
computes the full [64, 32] slice of the output; host concatenates.

Device algorithm per core (all matmuls bf16, fp32 accumulate):
  - satN [2048,128] fp32 loaded token-major -> sum-of-squares (ACT square +
    DVE reduce) -> sqrt (ACT) -> reciprocal (DVE) -> per-token scale
    (GpSimd tensor_scalar, casting to bf16) -> PE transpose ->
    satT [128d, 2048tok] bf16.
  - panoT [128d, 4096tok] bf16 arrives pre-transposed (host layout prep);
    pano is NOT normalized before the matmul: 1/|pano_i| is folded in after
    the j-max (max_j <p, s_j>/|p| == (max_j <p, s_j>) / |p|).
    pano norms: ACT square panoT -> 32 matmuls against ones -> sqrt -> recip.
  - main: per pano pair p (2 panos = 128 PSUM rows), 4 matmuls
    [128d,128tok]^T @ [128d,512] -> PSUM sim tile [128, 4x512] fp32.
    j-max: interleaved mix of (a) DVE reduce_max straight from PSUM and
    (b) ScalarE copy to SBUF bf16 + pairwise-max tree (DVE top level at
    bf16 2x, GpSimd tail levels) to spread work across all three engines.
  - epilogue (two halves): maxes * pscale (broadcast via step-0 AP), then a
    [128,2] block-ones matmul sums the 64 pano tokens -> [2,512] PSUM
    -> SBUF -> DMA out [64, 32].
"""

import numpy as np

N_CORES = 8
A, I, D = 64, 64, 128          # panos, pano tokens, dim
B, J = 256, 64                 # sats, sat tokens
B_SH = B // N_CORES            # 32 sats per core
TOK_SAT = B_SH * J             # 2048 sat tokens per core
TOK_PANO = A * I               # 4096 pano tokens
PAIRS = A // 2                 # 32 pano pairs
GROUPS = 4                     # sat groups of 512 tokens (8 sats) each

# Main-loop split: tree-pairs run in consecutive runs of TREE_RUN followed by
# DIRECT_RUN direct-reduce pairs, repeating. Tail pairs are tree.
DIRECT_RUN = 1
TREE_RUN = 2
N_DIRECT = 12                  # total direct pairs out of 32
TREE_BATCH = 2                 # tree-pairs per ScalarE-copy/tree batch
# NOTE: walrus ISA checks reject TensorTensor on the Pool engine (only
# add/mult fp32 exist in the Q7 ucode, no max) and reject DVE TensorTensor
# with both operands in PSUM — so the max tree runs entirely on the DVE and
# GpSimd only handles tensor_scalar work.
GPS_SQ_PANO = False            # pano squaring on ScalarE (Pool TT bf16 illegal)

_CACHE = {}


LEAD_D = 2                     # direct pairs at the very start (ACT busy w/ prep)
TAIL_D = 3                     # direct pairs at the very end (shorten tail)


def _pair_schedule():
    """Return list of (pair, is_direct): LEAD_D directs first, TAIL_D last,
    the rest spread in runs; tree pairs always in consecutive runs so
    batches stay contiguous in maxsb."""
    mid_d = max(N_DIRECT - LEAD_D - TAIL_D, 0)
    kinds = ["D"] * LEAD_D
    directs_left = mid_d
    i = len(kinds)
    n_mid = PAIRS - TAIL_D
    while i < n_mid:
        trees = min(TREE_RUN, n_mid - i - directs_left)
        for _ in range(trees):
            kinds.append("T")
            i += 1
        if directs_left > 0 and i < n_mid:
            kinds.append("D")
            directs_left -= 1
            i += 1
    kinds += ["D"] * TAIL_D
    return [(i, k == "D") for i, k in enumerate(kinds)]


def _build_nc(repeat=1):
    import concourse.bass as bass
    import concourse.bacc as bacc
    import concourse.tile as tile
    from concourse import mybir

    f32 = mybir.dt.float32
    bf16 = mybir.dt.bfloat16

    # Bacc (not Bass): its compile() runs generate_event_semaphores /
    # move_matmul_waits_to_ldweights, which split multi-sem waits that
    # walrus codegen rejects ("Too many sync wait commands").
    nc = bacc.Bacc("TRN2", target_bir_lowering=False, debug=False)
    satN_d = nc.declare_dram_parameter("satN", [TOK_SAT, D], f32, isOutput=False)
    panoT_d = nc.declare_dram_parameter("panoT", [D, TOK_PANO], bf16, isOutput=False)
    consts_d = nc.declare_dram_parameter("consts", [128, 131], bf16, isOutput=False)
    out_d = nc.declare_dram_parameter("out", [A, B_SH], f32, isOutput=True)

    with tile.TileContext(nc) as tc:
     for _rep in range(repeat):
        with (
            tc.tile_pool(name="persist", bufs=1) as persist,
        ):
            NT = TOK_SAT // 128          # 16 sat tiles of 128 tokens
            NH = NT // 2                 # tiles per half-chunk

            satT = persist.tile([128, TOK_SAT], bf16)
            panoT_sb = persist.tile([128, TOK_PANO], bf16)
            consts_sb = persist.tile([128, 132], bf16)
            maxsb = persist.tile([128, PAIRS * B_SH], bf16)   # [128, 1024]
            pscale = persist.tile([128, PAIRS], f32)
            scaled = persist.tile([128, PAIRS * B_SH], bf16)
            out_sb = persist.tile([2, PAIRS * B_SH], f32)

            satN_r = satN_d[:].rearrange("(t p) d -> p t d", p=128)

            with (
                tc.tile_pool(name="prep", bufs=1) as prep,
                tc.tile_pool(name="prep_psum", bufs=1, space="PSUM") as ppsum,
            ):
                satN_sb = prep.tile([128, NT, D], f32)
                sq_sat = prep.tile([128, NT, D], f32)
                nrm_sat = prep.tile([128, NT], f32)
                sscale = prep.tile([128, NT], f32)
                satn_bf = prep.tile([128, NT, D], bf16)

                # sat DMA + normalize, in 2 pipelined half-chunks
                for h in range(2):
                    tl = slice(NH * h, NH * (h + 1))
                    nc.sync.dma_start(out=satN_sb[:, tl, :], in_=satN_r[:, tl, :])
                    nc.scalar.square(sq_sat[:, tl, :], satN_sb[:, tl, :])
                    nc.vector.reduce_sum(
                        out=nrm_sat[:, tl],
                        in_=sq_sat[:, tl, :],
                        axis=mybir.AxisListType.X,
                    )
                    nc.scalar.sqrt(nrm_sat[:, tl], nrm_sat[:, tl])
                    nc.vector.reciprocal(sscale[:, tl], nrm_sat[:, tl])
                    for t in range(NH * h, NH * (h + 1)):
                        nc.gpsimd.tensor_scalar_mul(
                            satn_bf[:, t, :], satN_sb[:, t, :], sscale[:, t : t + 1]
                        )

                nc.sync.dma_start(out=consts_sb[:, 0:131], in_=consts_d[:])
                identity = consts_sb[:, 0:128]
                ones_blk = consts_sb[:, 128:130]
                ones1 = consts_sb[:, 130:131]
                nc.sync.dma_start(out=panoT_sb, in_=panoT_d[:])

                # PE transpose 16x [128,128] -> satT columns
                for h in range(2):
                    ps_tr = ppsum.tile([128, 1024], bf16, tag="tr", name=f"ps_tr_{h}")
                    for u in range(8):
                        t = 8 * h + u
                        nc.tensor.transpose(
                            ps_tr[:, 128 * u : 128 * (u + 1)],
                            satn_bf[:, t, :],
                            identity,
                        )
                    nc.scalar.copy(satT[:, 1024 * h : 1024 * (h + 1)], ps_tr)

            # ---------------- main loop ------------------------------
            with (
                tc.tile_pool(name="treep", bufs=2) as treep,
                tc.tile_pool(name="sim_psum", bufs=3, space="PSUM") as spsum,
                tc.tile_pool(name="s2_psum", bufs=1, space="PSUM") as s2psum,
            ):
                # pano norms: square -> 32 matmuls vs ones -> sqrt -> recip.
                # ps_n lives in the s2 pool (coexists with sim psum) so the
                # main loop does not wait on it.
                sq_pano = persist.tile([128, TOK_PANO], bf16)
                nc.scalar.square(sq_pano, panoT_sb)
                ps_n = s2psum.tile([128, PAIRS], f32, tag="ps_n")
                for p in range(PAIRS):
                    nc.tensor.matmul(
                        ps_n[:, p : p + 1],
                        sq_pano[:, 128 * p : 128 * (p + 1)],
                        ones1,
                        start=True,
                        stop=True,
                    )
                nrm_pano = persist.tile([128, PAIRS], f32)
                nc.scalar.sqrt(nrm_pano, ps_n)
                nc.vector.reciprocal(pscale, nrm_pano)
                def mm_half(ps, p, hh):
                    # hh-th half of pair p: groups 2*hh, 2*hh+1
                    for g in (2 * hh, 2 * hh + 1):
                        nc.tensor.matmul(
                            ps[:, 512 * (g - 2 * hh) : 512 * (g - 2 * hh + 1)],
                            panoT_sb[:, 128 * p : 128 * (p + 1)],
                            satT[:, 512 * g : 512 * (g + 1)],
                            start=True,
                            stop=True,
                        )

                def flush_tree(queue):
                    if not queue:
                        return
                    nb = len(queue)
                    p0 = queue[0][0]
                    simcp = treep.tile(
                        [128, TREE_BATCH * 2048], bf16, tag="simcp",
                        name=f"simcp_{p0}",
                    )
                    for u, (q, ps_a, ps_b) in enumerate(queue):
                        nc.scalar.copy(simcp[:, 2048 * u : 2048 * u + 1024], ps_a)
                        nc.scalar.copy(
                            simcp[:, 2048 * u + 1024 : 2048 * (u + 1)], ps_b
                        )
                    # pairwise-max tree over j (innermost 64), DVE bf16 2x
                    w = 32
                    src = simcp[:, : nb * 2048].rearrange("p (q j) -> p q j", j=J)
                    while w >= 1:
                        eng = nc.vector
                        if w > 1:
                            dst_t = treep.tile(
                                [128, TREE_BATCH * 32 * w], bf16, tag=f"tree{w}",
                                name=f"tree{w}_{p0}",
                            )
                            dst = dst_t[:, : nb * 32 * w].rearrange(
                                "p (q j) -> p q j", j=w
                            )
                        else:
                            dst = maxsb[:, B_SH * p0 : B_SH * (p0 + nb), None]
                        eng.tensor_tensor(
                            out=dst,
                            in0=src[:, :, 0:w],
                            in1=src[:, :, w : 2 * w],
                            op=mybir.AluOpType.max,
                        )
                        if w > 1:
                            src = dst
                        w //= 2
                    queue.clear()

                def stage2_half(h):
                    nc.vector.tensor_tensor(
                        out=scaled[:, 512 * h : 512 * (h + 1)].rearrange(
                            "p (q s) -> p q s", s=B_SH
                        ),
                        in0=maxsb[:, 512 * h : 512 * (h + 1)].rearrange(
                            "p (q s) -> p q s", s=B_SH
                        ),
                        in1=pscale[:, 16 * h : 16 * (h + 1), None].to_broadcast(
                            [128, 16, B_SH]
                        ),
                        op=mybir.AluOpType.mult,
                    )
                    ps2 = s2psum.tile([2, 512], f32, tag="ps2", name=f"ps2_{h}")
                    nc.tensor.matmul(
                        ps2,
                        ones_blk,
                        scaled[:, 512 * h : 512 * (h + 1)],
                        start=True,
                        stop=True,
                    )
                    nc.scalar.copy(out_sb[:, 512 * h : 512 * (h + 1)], ps2)

                queue = []
                done = 0
                for p, is_direct in _pair_schedule():
                    ps_a = spsum.tile([128, 1024], f32, tag="sim", name=f"psa_{p}")
                    mm_half(ps_a, p, 0)
                    ps_b = spsum.tile([128, 1024], f32, tag="sim", name=f"psb_{p}")
                    mm_half(ps_b, p, 1)
                    if is_direct:
                        nc.vector.reduce_max(
                            out=maxsb[:, B_SH * p : B_SH * p + 16],
                            in_=ps_a.rearrange("p (g s j) -> p g s j", g=2, j=J),
                            axis=mybir.AxisListType.X,
                        )
                        nc.vector.reduce_max(
                            out=maxsb[:, B_SH * p + 16 : B_SH * (p + 1)],
                            in_=ps_b.rearrange("p (g s j) -> p g s j", g=2, j=J),
                            axis=mybir.AxisListType.X,
                        )
                    else:
                        if queue and queue[-1][0] != p - 1:
                            flush_tree(queue)  # keep batches contiguous
                        queue.append((p, ps_a, ps_b))
                        if len(queue) == TREE_BATCH:
                            flush_tree(queue)
                    done += 1
                    if done == 16:
                        flush_tree(queue)
                        stage2_half(0)
                flush_tree(queue)
                stage2_half(1)

                nc.sync.dma_start(
                    out=out_d[:].rearrange("(q r) s -> r q s", r=2),
                    in_=out_sb.rearrange("r (q s) -> r q s", s=B_SH),
                )

    nc.finalize()  # Bacc: runs compile() (reg alloc + wait legalization)
    return nc


def _prep_inputs(sat, pano):
    """Host-side shard + layout prep. Returns per-core input maps."""
    import ml_dtypes

    bf16 = ml_dtypes.bfloat16
    pano = np.ascontiguousarray(pano, dtype=np.float32)
    sat = np.ascontiguousarray(sat, dtype=np.float32)

    panoT = np.ascontiguousarray(
        pano.reshape(TOK_PANO, D).T.astype(bf16)
    )  # [128, 4096]
    consts = np.zeros((128, 131), dtype=bf16)
    consts[:, 0:128] = np.eye(128, dtype=bf16)
    consts[0:64, 128] = bf16(1.0)
    consts[64:128, 129] = bf16(1.0)
    consts[:, 130] = bf16(1.0)

    in_maps = []
    for c in range(N_CORES):
        satN = np.ascontiguousarray(
            sat[c * B_SH : (c + 1) * B_SH].reshape(TOK_SAT, D)
        )
        in_maps.append({"satN": satN, "panoT": panoT, "consts": consts})
    return in_maps


def kernel(sat_embeddings_unnormalized, pano_embeddings_unnormalized):
    from concourse.bass_utils import run_bass_kernel_spmd

    if "nc" not in _CACHE:
        _CACHE["nc"] = _build_nc()
    nc = _CACHE["nc"]

    in_maps = _prep_inputs(
        np.asarray(sat_embeddings_unnormalized),
        np.asarray(pano_embeddings_unnormalized),
    )
    res = run_bass_kernel_spmd(nc, in_maps, list(range(N_CORES)))
    outs = [np.asarray(res.results[c]["out"], dtype=np.float32) for c in range(N_CORES)]
    return np.concatenate(outs, axis=1)  # [64, 256]
